# revision 33
# baseline (speedup 1.0000x reference)
"""CapsuleLayer (dynamic routing) Trainium2 kernel, 8-core data parallel.

Math (reference):
    u_hat[b,j,i,k] = sum_l W[j,i,k,l] * x[b,i,l]
    b_logits = 0; 3 routing iters:
        c = softmax_j(b_logits); s[b,j,k] = sum_i c[b,j,i]*u_hat[b,j,i,k]
        v = squash(s); b_logits += sum_k u_hat*v    (last iter: no update)
Key identity used: b_logits at iter r = sum_k u_hat[b,j,i,k] * V_r[b,j,k]
with V_r = v_0 + ... + v_{r-1}  (cumulative), since b_logits starts at 0.

Per-core layout (B_loc=16 batches, octs of 8):
    u_hat tiles U1[g=(oct*IG+ig)]: [p=(b8*16+i_sub16)=128, f=(k16*32+j32)=512] bf16
    generated by PE: lhsT = block-diag x  [(i_sub,l)=128, (b,i_sub')=128]
                     rhs  = W_re[ig]      [(i_sub,l)=128, (k,j)=512]
    b-pass:  pr = U1 * V1 (broadcast over g) ; sum_k via strided add tree
             (split between Vector and GpSimd engines)
    s-pass:  sp = U1 * c (c broadcast over k) ; sum_i via PE (block-diag DB)
    iter0:   s_0 = (sum_i u_hat)/32 from a single accumulated PE matmul (XT)
    squash:  both octs batched in one [16, F] chain; V broadcast via DMA
"""

import os
import numpy as np
import ml_dtypes

B, J, I, K, L = 128, 32, 1152, 16, 8
NC = 8              # cores
BL = B // NC        # 16 batches per core
IG = I // 16        # 72 i-groups of 16
NG = 2 * IG         # 144 u_hat tiles per core
F = K * J           # 512 free (k-major, j-inner)
EPS = 1e-7
N_ROUTINGS = 3
WCH = 4             # i-groups per W DMA chunk (4KB rows -> fewer descriptors)
NCH = IG // WCH     # 18 chunks
WCH8 = 4            # i-groups per fp8 W prefetch chunk (2KB rows)
NCH8 = IG // WCH8   # 18 chunks
GB = 6              # i-groups per DVE block
NB = IG // GB       # 12 blocks per oct

# engine assignment for routing work (per oct): block numbers given to GpSimd
# (empirically: gpsimd TT contends for the shared SBUF port and slows Vector
#  ~2.5x while running -- keep everything on Vector)
GP_B_BLOCKS = ()
GP_S_BLOCKS = ()

BF16 = ml_dtypes.bfloat16
FP8 = ml_dtypes.float8_e4m3

_last_exec_ns = None


def _build_nc(reps=1, skip=()):
    import concourse.bass as bass
    import concourse.tile as tile
    from concourse import mybir

    nc = bass.Bass()
    dt = mybir.dt

    eps_t = nc.alloc_sbuf_tensor("const-eps", [128, 1], dt.float32)
    nc.gpsimd.memset(eps_t.ap(), EPS)
    nc.const_aps.aps[(dt.float32, EPS)] = eps_t.ap()
    nc.all_engine_barrier()

    w_re = nc.declare_dram_parameter("w_re", [NCH, 128, WCH * F], dt.bfloat16,
                                     isOutput=False)
    x_bd = nc.declare_dram_parameter("x_bd", [NCH, 128, 2 * WCH * 128], dt.bfloat16,
                                     isOutput=False)
    xt_d = nc.declare_dram_parameter("xt", [128, IG * BL], dt.bfloat16,
                                     isOutput=False)
    db_d = nc.declare_dram_parameter("delta_b", [128, 8], dt.bfloat16,
                                     isOutput=False)
    d2_d = nc.declare_dram_parameter("delta_2", [8, 128], dt.bfloat16,
                                     isOutput=False)
    v_out = nc.declare_dram_parameter("v_out", [BL, F], dt.float32, isOutput=True)

    with nc.allow_low_precision(reason="deliberate bf16 storage"), \
         tile.TileContext(nc) as tc:
        with (
            tc.tile_pool(name="singles", bufs=1) as singles,
            tc.tile_pool(name="wpool", bufs=2) as wpool,
            tc.tile_pool(name="xpool", bufs=2) as xpool,

            tc.tile_pool(name="pu", bufs=4, space="PSUM") as pu,
            tc.tile_pool(name="pv", bufs=1, space="PSUM") as pv,
            tc.tile_pool(name="pm", bufs=1, space="PSUM") as pm,
            tc.tile_pool(name="ps", bufs=1, space="PSUM") as ps,
            tc.tile_pool(name="prpv", bufs=2) as prpv,
            tc.tile_pool(name="smalls", bufs=1) as smalls,
        ):
            # ---- resident SBUF tensors ----
            U1 = singles.tile([128, NG, F], dt.bfloat16)
            XT = singles.tile([128, IG, BL], dt.bfloat16)
            DB = singles.tile([128, 8], dt.bfloat16)
            D2 = singles.tile([8, 128], dt.bfloat16)
            LOG = singles.tile([128, 2, IG, J], dt.bfloat16)
            V1S = singles.tile([128, 2, F], dt.bfloat16)
            DEN = singles.tile([128, 2, IG], dt.float32)
            RDE = singles.tile([128, 2, IG], dt.float32)
            DBR = singles.tile([128, 2, IG, 8], dt.bfloat16)  # (1/den)-weighted DB
            VSB = singles.tile([8, 2, F], dt.bfloat16)   # cumulative V, bf16
            VCU = singles.tile([8, 2, F], dt.float32)    # current v_r

            nc.sync.dma_start(out=XT, in_=xt_d[:, :].rearrange("p (a b) -> p a b", a=IG))
            nc.sync.dma_start(out=DB, in_=db_d[:, :])
            nc.sync.dma_start(out=D2, in_=d2_d[:, :])

            for _rep in range(reps):
                # ---- P0: prefetch pass over W (bf16) -> PM = sum_i u_hat.
                # W is streamed twice (again in P1); this pass unblocks v0 so
                # Vector can run iter-1's b-pass under the generation phase.
                PM = pm.tile([BL, F], dt.float32)
                for ch8 in range(NCH):
                    w8 = wpool.tile([128, WCH, F], dt.bfloat16)
                    src8 = w_re[ch8, :, :].rearrange("p (a f) -> p a f", a=WCH)
                    nc.gpsimd.dma_start(out=w8[0:64], in_=src8[0:64])
                    nc.sync.dma_start(out=w8[64:128], in_=src8[64:128])
                    for igc in range(WCH):
                        ig = ch8 * WCH + igc
                        nc.tensor.matmul(PM, lhsT=XT[:, ig, :], rhs=w8[:, igc, :],
                                         start=(ig == 0), stop=(ig == IG - 1))

                # ---- squash: v = s*scale(s); s [8, (k j)] f32 (PSUM ok) ----
                def squash(src_ap, out_ap, pre_scale):
                    # sq = (s*pre)^2
                    sq = smalls.tile([8, F], dt.float32, tag="sq_sq")
                    nc.scalar.activation(sq, src_ap,
                                         mybir.ActivationFunctionType.Square,
                                         scale=pre_scale)
                    # s2[j] = sum_k sq[k,j]
                    s2 = smalls.tile([8, J], dt.float32, tag="sq_s2")
                    nc.vector.tensor_reduce(
                        s2, sq.rearrange("p (k j) -> p j k", k=K),
                        axis=mybir.AxisListType.X, op=mybir.AluOpType.add)
                    # r2 = 1/sqrt(s2+eps) = exp(-0.5*ln(s2+eps)); r1 = 1/(1+s2)
                    # (ln+exp share one act table set; Sqrt would thrash tables)
                    q1 = smalls.tile([8, J], dt.float32, tag="sq_q1")
                    nc.scalar.activation(q1, s2, mybir.ActivationFunctionType.Ln,
                                         bias=EPS)
                    t1 = smalls.tile([8, J], dt.float32, tag="sq_t1")
                    nc.scalar.add(t1, s2, 1.0)
                    r1 = smalls.tile([8, J], dt.float32, tag="sq_r1")
                    nc.vector.reciprocal(r1, t1)
                    r2 = smalls.tile([8, J], dt.float32, tag="sq_r2")
                    nc.scalar.activation(r2, q1, mybir.ActivationFunctionType.Exp,
                                         scale=-0.5)
                    # sc = s2 * r1 * r2 * pre
                    sc = smalls.tile([8, J], dt.float32, tag="sq_sc")
                    nc.vector.tensor_mul(sc, s2, r1)
                    nc.vector.tensor_mul(sc, sc, r2)
                    if pre_scale != 1.0:
                        nc.vector.tensor_scalar_mul(sc, sc, pre_scale)
                    # v = s * sc (broadcast over k)
                    nc.vector.tensor_mul(
                        out_ap.rearrange("p (a j) -> p a j", a=K),
                        src_ap.rearrange("p (a j) -> p a j", a=K),
                        sc[:, None, :].broadcast_to([8, K, J]))

                def v1s_broadcast():
                    for oct in (0, 1):
                        pvt = pv.tile([128, F], dt.float32, tag="pv")
                        nc.tensor.matmul(pvt, lhsT=D2, rhs=VSB[:, oct, :],
                                         start=True, stop=True)
                        nc.scalar.copy(V1S[:, oct, :], pvt)

                # ---- iter 0: v0 = squash(PM/32); VSB = v0 ----
                # DMA straight from PSUM on the (idle) vector engine's trigger
                # so the scheduler cannot sink it behind the gen DMA stream.
                S32 = smalls.tile([BL, F], dt.float32, tag="s32")
                nc.scalar.copy(S32, PM)
                SQ0 = smalls.tile([8, 2, F], dt.float32, tag="sq0")
                nc.scalar.dma_start(out=SQ0[:, 0, :], in_=S32[0:8, :])
                nc.scalar.dma_start(out=SQ0[:, 1, :], in_=S32[8:16, :])
                for oct in (0, 1):
                    squash(SQ0[:, oct, :], VCU[:, oct, :], 1.0 / J)
                nc.vector.tensor_copy(VSB, VCU)
                v1s_broadcast()

                # ---- P1: u_hat generation (bf16); all casts on Scalar so
                # Vector is free to run iter-1's b-pass as tiles land ----
                for ch in range(NCH):
                    wt = wpool.tile([128, WCH, F], dt.bfloat16)
                    srcw = w_re[ch, :, :].rearrange("p (a f) -> p a f", a=WCH)
                    nc.gpsimd.dma_start(out=wt[0:64], in_=srcw[0:64])
                    nc.sync.dma_start(out=wt[64:128], in_=srcw[64:128])
                    xb = xpool.tile([128, 2 * WCH, 128], dt.bfloat16)
                    srcx = x_bd[ch, :, :].rearrange("p (a f) -> p a f", a=2 * WCH)
                    nc.gpsimd.dma_start(out=xb[0:64], in_=srcx[0:64])
                    nc.sync.dma_start(out=xb[64:128], in_=srcx[64:128])
                    for igc in range(WCH):
                        ig = ch * WCH + igc
                        for oct in (0, 1):
                            if "gen" in skip:
                                continue
                            g = oct * IG + ig
                            put = pu.tile([128, F], dt.float32)
                            nc.tensor.matmul(put, lhsT=xb[:, igc * 2 + oct, :],
                                             rhs=wt[:, igc, :], start=True, stop=True)
                            nc.scalar.copy(U1[:, g, :], put)

                exp_f = mybir.ActivationFunctionType.Exp
                for r in range(1, N_ROUTINGS):
                    # ---- b-pass: logits = sum_k U1*V1 ----
                    for oct in (0, 1):
                        if "b" in skip:
                            nc.vector.memset(LOG[:, oct], 0.1)
                            continue
                        for bn in range(NB):
                            igb = bn * GB
                            eng = nc.vector
                            pool = prpv
                            u4 = U1[:, oct * IG + igb:oct * IG + igb + GB, :]
                            pr = pool.tile([128, GB, F], dt.bfloat16, tag="bprod")
                            eng.tensor_mul(
                                pr, u4,
                                V1S[:, oct, None, :].broadcast_to([128, GB, F]))
                            eng.tensor_add(pr[:, :, 0:256], pr[:, :, 0:256],
                                           pr[:, :, 256:512])
                            eng.tensor_add(pr[:, :, 0:128], pr[:, :, 0:128],
                                           pr[:, :, 128:256])
                            eng.tensor_add(pr[:, :, 0:64], pr[:, :, 0:64],
                                           pr[:, :, 64:128])
                            eng.tensor_add(LOG[:, oct, igb:igb + GB, :],
                                           pr[:, :, 0:32], pr[:, :, 32:64])
                    # ---- softmax over j: e=exp(logits); 1/den folded into DBR ----
                    for oct in (0, 1):
                        if "sm" in skip:
                            nc.vector.memset(LOG[:, oct], 0.03)
                            continue
                        nc.scalar.activation(LOG[:, oct], LOG[:, oct], exp_f)
                        nc.vector.tensor_reduce(DEN[:, oct], LOG[:, oct],
                                                axis=mybir.AxisListType.X,
                                                op=mybir.AluOpType.add)
                        nc.vector.reciprocal(RDE[:, oct], DEN[:, oct])
                        # DBR[p, ig, b'] = DB[p, b'] * (1/den)[p, ig]
                        nc.vector.tensor_mul(
                            DBR[:, oct],
                            DB[:, None, :].broadcast_to([128, IG, 8]),
                            RDE[:, oct, :, None].broadcast_to([128, IG, 8]))
                    # ---- s-pass ----
                    PS0 = ps.tile([8, F], dt.float32, tag="ps0")
                    PS1 = ps.tile([8, F], dt.float32, tag="ps1")
                    PSo = (PS0, PS1)
                    for oct in (0, 1):
                        for bn in range(NB):
                            igb = bn * GB
                            if "s" in skip and igb > 0:
                                continue
                            eng = nc.vector
                            pool = prpv
                            u4 = U1[:, oct * IG + igb:oct * IG + igb + GB, :]
                            sp = pool.tile([128, GB, K, J], dt.bfloat16, tag="bprod")
                            eng.tensor_mul(
                                sp,
                                u4.rearrange("p a (k j) -> p a k j", k=K),
                                LOG[:, oct, igb:igb + GB, None, :]
                                .broadcast_to([128, GB, K, J]))
                            for i4 in range(GB):
                                nc.tensor.matmul(
                                    PSo[oct], lhsT=DBR[:, oct, igb + i4, :],
                                    rhs=sp[:, i4, :, :],
                                    start=(igb == 0 and i4 == 0),
                                    stop=(igb + GB == IG and i4 == GB - 1)
                                    if "s" not in skip else
                                    (igb == 0 and i4 == GB - 1))
                    # ---- squash per oct ----
                    for oct in (0, 1):
                        squash(PSo[oct][:, :], VCU[:, oct, :], 1.0)
                    if r < N_ROUTINGS - 1:
                        nc.vector.tensor_add(VSB, VSB, VCU)
                        v1s_broadcast()

                nc.sync.dma_start(out=v_out[:, :].rearrange("(a b) f -> b a f", a=2),
                                  in_=VCU)
    _split_multi_waits(nc, mybir)
    return nc


def _split_multi_waits(nc, mybir):
    """This walrus build allows at most one sync-wait per instruction;
    hoist extra waits into preceding same-engine NOPs."""
    for bb in nc.main_func.blocks:
        i = 0
        while i < len(bb.instructions):
            inst = bb.instructions[i]
            si = inst.sync_info
            ow = (si.on_wait if si else None) or []
            if len(ow) > 1:
                for w in ow[:-1]:
                    nop = mybir.InstNoOp(name=nc.get_next_instruction_name(),
                                         ins=[], outs=[])
                    nop.engine = inst.engine
                    nop.sync_info = mybir.SyncInfo(on_wait=[w], on_update=[])
                    nc.register_instruction(nop)
                    bb.instructions.insert(i, nop)
                    i += 1
                si.on_wait = [ow[-1]]
            i += 1


def _host_prep(inputs, W):
    """Returns per-core input maps."""
    # W_re[ig, isub*8+l, k*32+j] = W[j, 16*ig+isub, k, l]
    Wr = W.reshape(J, IG, 16, K, L)
    W_re = np.ascontiguousarray(Wr.transpose(1, 2, 4, 3, 0))   # [ig,isub,l,k,j]
    W_re = W_re.reshape(IG, 128, F)
    # chunk-major for DMA: [NCH, 128, WCH*F]
    W_ch = np.ascontiguousarray(W_re.reshape(NCH, WCH, 128, F)
                                .transpose(0, 2, 1, 3)).reshape(NCH, 128, WCH * F)
    W_ch = W_ch.astype(BF16)

    db = np.zeros((128, 8), np.float32)
    for b in range(8):
        db[b * 16:(b + 1) * 16, b] = 1.0
    d2 = np.zeros((8, 128), np.float32)
    for b in range(8):
        d2[b, b * 16:(b + 1) * 16] = 1.0

    per_core = []
    for cid in range(NC):
        xc = inputs[cid * BL:(cid + 1) * BL]          # [16, 1152, 8]
        xr = xc.reshape(2, 8, IG, 16, L)              # [oct,b,ig,isub,l]
        xbd = np.zeros((2, IG, 16, L, 8, 16), np.float32)
        t = xr.transpose(0, 2, 3, 4, 1)               # [oct,ig,isub,l,b]
        for s in range(16):
            xbd[:, :, s, :, :, s] = t[:, :, s, :, :]
        # -> [ig, oct, 128, 128]
        xbd = xbd.reshape(2, IG, 128, 128).transpose(1, 0, 2, 3)
        # chunk for DMA: [NCH, 128, 2*WCH*128]; order inside chunk: (igc, oct)
        xbd = np.ascontiguousarray(
            xbd.reshape(NCH, WCH * 2, 128, 128).transpose(0, 2, 1, 3)
        ).reshape(NCH, 128, 2 * WCH * 128).astype(BF16)

        xt = np.ascontiguousarray(
            xc.reshape(BL, IG, 16, L).transpose(1, 2, 3, 0)   # [ig,isub,l,b]
        ).reshape(IG, 128, BL).transpose(1, 0, 2)             # [128, ig, b]
        xt = np.ascontiguousarray(xt).reshape(128, IG * BL).astype(BF16)

        per_core.append({
            "w_re": W_ch,
            "x_bd": xbd,
            "xt": xt,
            "delta_b": db.astype(BF16),
            "delta_2": d2.astype(BF16),
        })
    return per_core


_nc_cache = None


def kernel(inputs, W):
    global _nc_cache, _last_exec_ns
    from concourse.bass_utils import run_bass_kernel_spmd

    inputs = np.asarray(inputs, dtype=np.float32)
    W = np.asarray(W, dtype=np.float32)
    if _nc_cache is None:
        _nc_cache = _build_nc()
    nc = _nc_cache
    in_maps = _host_prep(inputs, W)
    trace = bool(int(os.environ.get("CAPS_TRACE", "0")))
    res = run_bass_kernel_spmd(nc, in_maps, core_ids=list(range(NC)), trace=trace)
    _last_exec_ns = res.exec_time_ns
    out = np.empty((B, J, K), np.float32)
    for cid in range(NC):
        vo = res.results[cid]["v_out"].reshape(BL, K, J)      # [b,(k,j)]
        out[cid * BL:(cid + 1) * BL] = vo.transpose(0, 2, 1)  # -> [b,j,k]
    return out


# revision 34
# speedup vs baseline: 1.0341x; 1.0341x over previous
"""CapsuleLayer (dynamic routing) Trainium2 kernel, 8-core data parallel.

Math (reference):
    u_hat[b,j,i,k] = sum_l W[j,i,k,l] * x[b,i,l]
    b_logits = 0; 3 routing iters:
        c = softmax_j(b_logits); s[b,j,k] = sum_i c[b,j,i]*u_hat[b,j,i,k]
        v = squash(s); b_logits += sum_k u_hat*v    (last iter: no update)
Key identity used: b_logits at iter r = sum_k u_hat[b,j,i,k] * V_r[b,j,k]
with V_r = v_0 + ... + v_{r-1}  (cumulative), since b_logits starts at 0.

Per-core layout (B_loc=16 batches, octs of 8):
    u_hat tiles U1[g=(oct*IG+ig)]: [p=(b8*16+i_sub16)=128, f=(k16*32+j32)=512] bf16
    generated by PE: lhsT = block-diag x  [(i_sub,l)=128, (b,i_sub')=128]
                     rhs  = W_re[ig]      [(i_sub,l)=128, (k,j)=512]
    b-pass:  pr = U1 * V1 (broadcast over g) ; sum_k via strided add tree
             (split between Vector and GpSimd engines)
    s-pass:  sp = U1 * c (c broadcast over k) ; sum_i via PE (block-diag DB)
    iter0:   s_0 = (sum_i u_hat)/32 from a single accumulated PE matmul (XT)
    squash:  both octs batched in one [16, F] chain; V broadcast via DMA
"""

import os
import numpy as np
import ml_dtypes

B, J, I, K, L = 128, 32, 1152, 16, 8
NC = 8              # cores
BL = B // NC        # 16 batches per core
IG = I // 16        # 72 i-groups of 16
NG = 2 * IG         # 144 u_hat tiles per core
F = K * J           # 512 free (k-major, j-inner)
EPS = 1e-7
N_ROUTINGS = 3
WCH = 4             # i-groups per W DMA chunk (4KB rows -> fewer descriptors)
NCH = IG // WCH     # 18 chunks
WCH8 = 4            # i-groups per fp8 W prefetch chunk (2KB rows)
NCH8 = IG // WCH8   # 18 chunks
GB = 6              # i-groups per DVE block
NB = IG // GB       # 12 blocks per oct

# engine assignment for routing work (per oct): block numbers given to GpSimd
# (empirically: gpsimd TT contends for the shared SBUF port and slows Vector
#  ~2.5x while running -- keep everything on Vector)
GP_B_BLOCKS = ()
GP_S_BLOCKS = ()

BF16 = ml_dtypes.bfloat16
FP8 = ml_dtypes.float8_e4m3

_last_exec_ns = None


def _build_nc(reps=1, skip=()):
    import concourse.bass as bass
    import concourse.tile as tile
    from concourse import mybir

    nc = bass.Bass()
    dt = mybir.dt

    eps_t = nc.alloc_sbuf_tensor("const-eps", [128, 1], dt.float32)
    nc.gpsimd.memset(eps_t.ap(), EPS)
    nc.const_aps.aps[(dt.float32, EPS)] = eps_t.ap()
    nc.all_engine_barrier()

    w_re = nc.declare_dram_parameter("w_re", [NCH, 128, WCH * F], dt.bfloat16,
                                     isOutput=False)
    x_bd = nc.declare_dram_parameter("x_bd", [NCH, 128, 2 * WCH * 128], dt.bfloat16,
                                     isOutput=False)
    xt_d = nc.declare_dram_parameter("xt", [128, IG * BL], dt.bfloat16,
                                     isOutput=False)
    db_d = nc.declare_dram_parameter("delta_b", [128, 8], dt.bfloat16,
                                     isOutput=False)
    d2_d = nc.declare_dram_parameter("delta_2", [8, 128], dt.bfloat16,
                                     isOutput=False)
    v_out = nc.declare_dram_parameter("v_out", [BL, F], dt.float32, isOutput=True)

    with nc.allow_low_precision(reason="deliberate bf16 storage"), \
         tile.TileContext(nc) as tc:
        with (
            tc.tile_pool(name="singles", bufs=1) as singles,
            tc.tile_pool(name="wpool", bufs=2) as wpool,
            tc.tile_pool(name="xpool", bufs=2) as xpool,

            tc.tile_pool(name="pu", bufs=4, space="PSUM") as pu,
            tc.tile_pool(name="pv", bufs=1, space="PSUM") as pv,
            tc.tile_pool(name="pm", bufs=1, space="PSUM") as pm,
            tc.tile_pool(name="ps", bufs=1, space="PSUM") as ps,
            tc.tile_pool(name="prpv", bufs=2) as prpv,
            tc.tile_pool(name="smalls", bufs=1) as smalls,
        ):
            # ---- resident SBUF tensors ----
            U1 = singles.tile([128, NG, F], dt.bfloat16)
            XT = singles.tile([128, IG, BL], dt.bfloat16)
            DB = singles.tile([128, 8], dt.bfloat16)
            D2 = singles.tile([8, 128], dt.bfloat16)
            LOG = singles.tile([128, 2, IG, J], dt.bfloat16)
            V1S = singles.tile([128, 2, F], dt.bfloat16)
            DEN = singles.tile([128, 2, IG], dt.float32)
            RDE = singles.tile([128, 2, IG], dt.float32)
            DBR = singles.tile([128, 2, IG, 8], dt.bfloat16)  # (1/den)-weighted DB
            VSB = singles.tile([8, 2, F], dt.bfloat16)   # cumulative V, bf16
            VCU = singles.tile([8, 2, F], dt.float32)    # current v_r

            nc.sync.dma_start(out=XT, in_=xt_d[:, :].rearrange("p (a b) -> p a b", a=IG))
            nc.sync.dma_start(out=DB, in_=db_d[:, :])
            nc.sync.dma_start(out=D2, in_=d2_d[:, :])

            for _rep in range(reps):
                # ---- P0: prefetch pass over W (bf16) -> PM = sum_i u_hat.
                # W is streamed twice (again in P1); this pass unblocks v0 so
                # Vector can run iter-1's b-pass under the generation phase.
                # high_priority: the scheduler must not defer this chain behind
                # the gen stream -- v0 gates all of Vector's routing work.
                PM = pm.tile([BL, F], dt.float32)
                with tc.high_priority():
                    for ch8 in range(NCH):
                        w8 = wpool.tile([128, WCH, F], dt.bfloat16)
                        src8 = w_re[ch8, :, :].rearrange("p (a f) -> p a f", a=WCH)
                        nc.gpsimd.dma_start(out=w8[0:64], in_=src8[0:64])
                        nc.sync.dma_start(out=w8[64:128], in_=src8[64:128])
                        for igc in range(WCH):
                            ig = ch8 * WCH + igc
                            nc.tensor.matmul(PM, lhsT=XT[:, ig, :],
                                             rhs=w8[:, igc, :],
                                             start=(ig == 0), stop=(ig == IG - 1))

                # ---- squash: v = s*scale(s); s [8, (k j)] f32 (PSUM ok) ----
                def squash(src_ap, out_ap, pre_scale):
                    # sq = (s*pre)^2
                    sq = smalls.tile([8, F], dt.float32, tag="sq_sq")
                    nc.scalar.activation(sq, src_ap,
                                         mybir.ActivationFunctionType.Square,
                                         scale=pre_scale)
                    # s2[j] = sum_k sq[k,j]
                    s2 = smalls.tile([8, J], dt.float32, tag="sq_s2")
                    nc.vector.tensor_reduce(
                        s2, sq.rearrange("p (k j) -> p j k", k=K),
                        axis=mybir.AxisListType.X, op=mybir.AluOpType.add)
                    # r2 = 1/sqrt(s2+eps) = exp(-0.5*ln(s2+eps)); r1 = 1/(1+s2)
                    # (ln+exp share one act table set; Sqrt would thrash tables)
                    q1 = smalls.tile([8, J], dt.float32, tag="sq_q1")
                    nc.scalar.activation(q1, s2, mybir.ActivationFunctionType.Ln,
                                         bias=EPS)
                    t1 = smalls.tile([8, J], dt.float32, tag="sq_t1")
                    nc.scalar.add(t1, s2, 1.0)
                    r1 = smalls.tile([8, J], dt.float32, tag="sq_r1")
                    nc.vector.reciprocal(r1, t1)
                    r2 = smalls.tile([8, J], dt.float32, tag="sq_r2")
                    nc.scalar.activation(r2, q1, mybir.ActivationFunctionType.Exp,
                                         scale=-0.5)
                    # sc = s2 * r1 * r2 * pre
                    sc = smalls.tile([8, J], dt.float32, tag="sq_sc")
                    nc.vector.tensor_mul(sc, s2, r1)
                    nc.vector.tensor_mul(sc, sc, r2)
                    if pre_scale != 1.0:
                        nc.vector.tensor_scalar_mul(sc, sc, pre_scale)
                    # v = s * sc (broadcast over k)
                    nc.vector.tensor_mul(
                        out_ap.rearrange("p (a j) -> p a j", a=K),
                        src_ap.rearrange("p (a j) -> p a j", a=K),
                        sc[:, None, :].broadcast_to([8, K, J]))

                def v1s_broadcast():
                    for oct in (0, 1):
                        pvt = pv.tile([128, F], dt.float32, tag="pv")
                        nc.tensor.matmul(pvt, lhsT=D2, rhs=VSB[:, oct, :],
                                         start=True, stop=True)
                        nc.scalar.copy(V1S[:, oct, :], pvt)

                # ---- iter 0: v0 = squash(PM/32); VSB = v0 ----
                with tc.high_priority():
                    S32 = smalls.tile([BL, F], dt.float32, tag="s32")
                    nc.scalar.copy(S32, PM)
                    SQ0 = smalls.tile([8, 2, F], dt.float32, tag="sq0")
                    nc.scalar.dma_start(out=SQ0[:, 0, :], in_=S32[0:8, :])
                    nc.scalar.dma_start(out=SQ0[:, 1, :], in_=S32[8:16, :])
                    for oct in (0, 1):
                        squash(SQ0[:, oct, :], VCU[:, oct, :], 1.0 / J)
                    nc.vector.tensor_copy(VSB, VCU)
                    v1s_broadcast()

                # ---- P1: u_hat generation (bf16); all casts on Scalar so
                # Vector is free to run iter-1's b-pass as tiles land ----
                for ch in range(NCH):
                    wt = wpool.tile([128, WCH, F], dt.bfloat16)
                    srcw = w_re[ch, :, :].rearrange("p (a f) -> p a f", a=WCH)
                    nc.gpsimd.dma_start(out=wt[0:64], in_=srcw[0:64])
                    nc.sync.dma_start(out=wt[64:128], in_=srcw[64:128])
                    xb = xpool.tile([128, 2 * WCH, 128], dt.bfloat16)
                    srcx = x_bd[ch, :, :].rearrange("p (a f) -> p a f", a=2 * WCH)
                    nc.gpsimd.dma_start(out=xb[0:64], in_=srcx[0:64])
                    nc.sync.dma_start(out=xb[64:128], in_=srcx[64:128])
                    for igc in range(WCH):
                        ig = ch * WCH + igc
                        for oct in (0, 1):
                            if "gen" in skip:
                                continue
                            g = oct * IG + ig
                            put = pu.tile([128, F], dt.float32)
                            nc.tensor.matmul(put, lhsT=xb[:, igc * 2 + oct, :],
                                             rhs=wt[:, igc, :], start=True, stop=True)
                            nc.scalar.copy(U1[:, g, :], put)

                exp_f = mybir.ActivationFunctionType.Exp
                for r in range(1, N_ROUTINGS):
                    # ---- b-pass: logits = sum_k U1*V1 ----
                    for oct in (0, 1):
                        if "b" in skip:
                            nc.vector.memset(LOG[:, oct], 0.1)
                            continue
                        for bn in range(NB):
                            igb = bn * GB
                            eng = nc.vector
                            pool = prpv
                            u4 = U1[:, oct * IG + igb:oct * IG + igb + GB, :]
                            pr = pool.tile([128, GB, F], dt.bfloat16, tag="bprod")
                            eng.tensor_mul(
                                pr, u4,
                                V1S[:, oct, None, :].broadcast_to([128, GB, F]))
                            eng.tensor_add(pr[:, :, 0:256], pr[:, :, 0:256],
                                           pr[:, :, 256:512])
                            eng.tensor_add(pr[:, :, 0:128], pr[:, :, 0:128],
                                           pr[:, :, 128:256])
                            eng.tensor_add(pr[:, :, 0:64], pr[:, :, 0:64],
                                           pr[:, :, 64:128])
                            eng.tensor_add(LOG[:, oct, igb:igb + GB, :],
                                           pr[:, :, 0:32], pr[:, :, 32:64])
                    # ---- softmax over j: e=exp(logits); 1/den folded into DBR ----
                    for oct in (0, 1):
                        if "sm" in skip:
                            nc.vector.memset(LOG[:, oct], 0.03)
                            continue
                        nc.scalar.activation(LOG[:, oct], LOG[:, oct], exp_f)
                        nc.vector.tensor_reduce(DEN[:, oct], LOG[:, oct],
                                                axis=mybir.AxisListType.X,
                                                op=mybir.AluOpType.add)
                        nc.vector.reciprocal(RDE[:, oct], DEN[:, oct])
                        # DBR[p, ig, b'] = DB[p, b'] * (1/den)[p, ig]
                        nc.vector.tensor_mul(
                            DBR[:, oct],
                            DB[:, None, :].broadcast_to([128, IG, 8]),
                            RDE[:, oct, :, None].broadcast_to([128, IG, 8]))
                    # ---- s-pass ----
                    PS0 = ps.tile([8, F], dt.float32, tag="ps0")
                    PS1 = ps.tile([8, F], dt.float32, tag="ps1")
                    PSo = (PS0, PS1)
                    for oct in (0, 1):
                        for bn in range(NB):
                            igb = bn * GB
                            if "s" in skip and igb > 0:
                                continue
                            eng = nc.vector
                            pool = prpv
                            u4 = U1[:, oct * IG + igb:oct * IG + igb + GB, :]
                            sp = pool.tile([128, GB, K, J], dt.bfloat16, tag="bprod")
                            eng.tensor_mul(
                                sp,
                                u4.rearrange("p a (k j) -> p a k j", k=K),
                                LOG[:, oct, igb:igb + GB, None, :]
                                .broadcast_to([128, GB, K, J]))
                            for i4 in range(GB):
                                nc.tensor.matmul(
                                    PSo[oct], lhsT=DBR[:, oct, igb + i4, :],
                                    rhs=sp[:, i4, :, :],
                                    start=(igb == 0 and i4 == 0),
                                    stop=(igb + GB == IG and i4 == GB - 1)
                                    if "s" not in skip else
                                    (igb == 0 and i4 == GB - 1))
                    # ---- squash per oct ----
                    for oct in (0, 1):
                        squash(PSo[oct][:, :], VCU[:, oct, :], 1.0)
                    if r < N_ROUTINGS - 1:
                        nc.vector.tensor_add(VSB, VSB, VCU)
                        v1s_broadcast()

                nc.sync.dma_start(out=v_out[:, :].rearrange("(a b) f -> b a f", a=2),
                                  in_=VCU)
    _split_multi_waits(nc, mybir)
    return nc


def _split_multi_waits(nc, mybir):
    """This walrus build allows at most one sync-wait per instruction;
    hoist extra waits into preceding same-engine NOPs."""
    for bb in nc.main_func.blocks:
        i = 0
        while i < len(bb.instructions):
            inst = bb.instructions[i]
            si = inst.sync_info
            ow = (si.on_wait if si else None) or []
            if len(ow) > 1:
                for w in ow[:-1]:
                    nop = mybir.InstNoOp(name=nc.get_next_instruction_name(),
                                         ins=[], outs=[])
                    nop.engine = inst.engine
                    nop.sync_info = mybir.SyncInfo(on_wait=[w], on_update=[])
                    nc.register_instruction(nop)
                    bb.instructions.insert(i, nop)
                    i += 1
                si.on_wait = [ow[-1]]
            i += 1


def _host_prep(inputs, W):
    """Returns per-core input maps."""
    # W_re[ig, isub*8+l, k*32+j] = W[j, 16*ig+isub, k, l]
    Wr = W.reshape(J, IG, 16, K, L)
    W_re = np.ascontiguousarray(Wr.transpose(1, 2, 4, 3, 0))   # [ig,isub,l,k,j]
    W_re = W_re.reshape(IG, 128, F)
    # chunk-major for DMA: [NCH, 128, WCH*F]
    W_ch = np.ascontiguousarray(W_re.reshape(NCH, WCH, 128, F)
                                .transpose(0, 2, 1, 3)).reshape(NCH, 128, WCH * F)
    W_ch = W_ch.astype(BF16)

    db = np.zeros((128, 8), np.float32)
    for b in range(8):
        db[b * 16:(b + 1) * 16, b] = 1.0
    d2 = np.zeros((8, 128), np.float32)
    for b in range(8):
        d2[b, b * 16:(b + 1) * 16] = 1.0

    per_core = []
    for cid in range(NC):
        xc = inputs[cid * BL:(cid + 1) * BL]          # [16, 1152, 8]
        xr = xc.reshape(2, 8, IG, 16, L)              # [oct,b,ig,isub,l]
        xbd = np.zeros((2, IG, 16, L, 8, 16), np.float32)
        t = xr.transpose(0, 2, 3, 4, 1)               # [oct,ig,isub,l,b]
        for s in range(16):
            xbd[:, :, s, :, :, s] = t[:, :, s, :, :]
        # -> [ig, oct, 128, 128]
        xbd = xbd.reshape(2, IG, 128, 128).transpose(1, 0, 2, 3)
        # chunk for DMA: [NCH, 128, 2*WCH*128]; order inside chunk: (igc, oct)
        xbd = np.ascontiguousarray(
            xbd.reshape(NCH, WCH * 2, 128, 128).transpose(0, 2, 1, 3)
        ).reshape(NCH, 128, 2 * WCH * 128).astype(BF16)

        xt = np.ascontiguousarray(
            xc.reshape(BL, IG, 16, L).transpose(1, 2, 3, 0)   # [ig,isub,l,b]
        ).reshape(IG, 128, BL).transpose(1, 0, 2)             # [128, ig, b]
        xt = np.ascontiguousarray(xt).reshape(128, IG * BL).astype(BF16)

        per_core.append({
            "w_re": W_ch,
            "x_bd": xbd,
            "xt": xt,
            "delta_b": db.astype(BF16),
            "delta_2": d2.astype(BF16),
        })
    return per_core


_nc_cache = None


def kernel(inputs, W):
    global _nc_cache, _last_exec_ns
    from concourse.bass_utils import run_bass_kernel_spmd

    inputs = np.asarray(inputs, dtype=np.float32)
    W = np.asarray(W, dtype=np.float32)
    if _nc_cache is None:
        _nc_cache = _build_nc()
    nc = _nc_cache
    in_maps = _host_prep(inputs, W)
    trace = bool(int(os.environ.get("CAPS_TRACE", "0")))
    res = run_bass_kernel_spmd(nc, in_maps, core_ids=list(range(NC)), trace=trace)
    _last_exec_ns = res.exec_time_ns
    out = np.empty((B, J, K), np.float32)
    for cid in range(NC):
        vo = res.results[cid]["v_out"].reshape(BL, K, J)      # [b,(k,j)]
        out[cid * BL:(cid + 1) * BL] = vo.transpose(0, 2, 1)  # -> [b,j,k]
    return out
